# revision 5
# baseline (speedup 1.0000x reference)
"""AttentionPooling (segment softmax-pool) Trainium2 kernel.

out[s,:] = sum_n 1[idx[n]==s] * gnorm[n] * (x[n,:] @ msg_w + msg_b)
  gnorm[n] = w[n]^p * exp(gate[n]) / (denom[seg] + eps)   (max-sub skipped:
  mathematically identical after normalization, logits are O(5))

Restructured so the big matmul contracts rows via a one-hot:
  A[s,d]   = sum_n G[n,s] * x[n,d],  denom[s] = sum_n G[n,s]   (ones col)
  out[s,:] = (A[s,:] @ msg_w) / (denom+eps) + (denom/(denom+eps)) * msg_b
where G[n,s] = 1[idx[n]==s] * g[n] is built per 128-row tile with one fused
DVE tensor_scalar(is_equal, mult) against an iota row.

Sharding: index is sorted; host assigns 2048 contiguous segments per core,
16 windows x 128 segments, rows of each window padded to 66*128 = 8448.

Engine assignment (v2): PE = A-matmul + phase2; DVE = G-build, logit reduce,
small ops, phase2 copies; GPSIMD = logit multiply; ACT = exp only (ln hoisted
to one pre-pass) so its LUT never reloads.
"""

import os
import sys
import numpy as np

for _p in ("/opt/trn_rl_repo", "/root/.axon_site/_ro/trn_rl_repo"):
    if os.path.isdir(_p) and _p not in sys.path:
        sys.path.insert(0, _p)

P = 128
S = 16384
D = 128
NCORES = 8
WIN = 128                      # segments per PSUM window
NWIN = S // WIN                # 128 global windows
NWIN_CORE = NWIN // NCORES     # 16 per core
TPW = 66                       # 128-row tiles per window (padded)
GROUP = 11                     # tiles per DMA/logit super-group
GPW = TPW // GROUP             # 6 groups per window
NT = NWIN_CORE * TPW           # 1056 tiles per core
NG = NT // GROUP               # 96 groups per core
ROWS_CORE = NT * P             # 135168 padded rows per core
EPS = 1e-10

IOTA_BF16 = True               # bf16 iota input for faster DVE G-build
MULT_ON_GPSIMD = True          # logit multiply on GPSIMD instead of DVE

LAST_EXEC_NS = None
LAST_RESULTS = None

_module_cache = {}


def _build_module():
    if "nc" in _module_cache:
        return _module_cache["nc"]

    import concourse.bass as bass  # noqa: F401
    import concourse.tile as tile
    from concourse import bacc, mybir
    from concourse.masks import make_identity

    f32 = mybir.dt.float32
    bf16 = mybir.dt.bfloat16
    iota_dt = bf16 if IOTA_BF16 else f32
    AX = mybir.AxisListType
    ALU = mybir.AluOpType
    ACTF = mybir.ActivationFunctionType

    nc = bacc.Bacc(
        "TRN2",
        target_bir_lowering=False,
        debug=False,
        enable_asserts=True,
        num_devices=NCORES,
    )

    xp = nc.dram_tensor("xp", [NG * P, GROUP * (D + 1)], f32, kind="ExternalInput")
    idxall = nc.dram_tensor("idxall", [P, NT], f32, kind="ExternalInput")
    wall = nc.dram_tensor("wall", [P, NT], f32, kind="ExternalInput")
    gwrep = nc.dram_tensor("gwrep", [P, GROUP * D], f32, kind="ExternalInput")
    msgw = nc.dram_tensor("msgw", [D, D], f32, kind="ExternalInput")
    msgbrep = nc.dram_tensor("msgbrep", [P, D], f32, kind="ExternalInput")
    gatebrep = nc.dram_tensor("gatebrep", [P, 1], f32, kind="ExternalInput")
    prep = nc.dram_tensor("prep", [P, 1], f32, kind="ExternalInput")
    iota = nc.dram_tensor("iota", [P, WIN], iota_dt, kind="ExternalInput")
    out = nc.dram_tensor("out", [NWIN_CORE * P, D], f32, kind="ExternalOutput")

    with tile.TileContext(nc) as tc:
        from contextlib import ExitStack

        with ExitStack() as ctx:
            const_pool = ctx.enter_context(tc.tile_pool(name="const", bufs=1))
            xs_pool = ctx.enter_context(tc.tile_pool(name="xs", bufs=6))
            grp_pool = ctx.enter_context(tc.tile_pool(name="grp", bufs=3))
            g_pool = ctx.enter_context(tc.tile_pool(name="gm", bufs=6))
            psA_pool = ctx.enter_context(tc.tile_pool(name="psA", bufs=2, space="PSUM"))
            ps2_pool = ctx.enter_context(tc.tile_pool(name="ps2", bufs=2, space="PSUM"))
            ph2_pool = ctx.enter_context(tc.tile_pool(name="ph2", bufs=2))

            gw_t = const_pool.tile([P, GROUP * D], f32)
            nc.sync.dma_start(gw_t[:], gwrep[:, :])
            msgw_t = const_pool.tile([D, D], f32)
            nc.sync.dma_start(msgw_t[:], msgw[:, :])
            msgb_t = const_pool.tile([P, D], f32)
            nc.sync.dma_start(msgb_t[:], msgbrep[:, :])
            gateb_t = const_pool.tile([P, 1], f32)
            nc.sync.dma_start(gateb_t[:], gatebrep[:, :])
            p_t = const_pool.tile([P, 1], f32)
            nc.sync.dma_start(p_t[:], prep[:, :])
            iota_t = const_pool.tile([P, WIN], iota_dt)
            nc.sync.dma_start(iota_t[:], iota[:, :])
            ident = const_pool.tile([P, P], f32)
            make_identity(nc, ident[:])

            # hoisted: all idx scalars + p*ln(w) for every tile in two ops
            idx_t = const_pool.tile([P, NT], f32)
            nc.sync.dma_start(idx_t[:], idxall[:, :])
            w_t = const_pool.tile([P, NT], f32)
            nc.sync.dma_start(w_t[:], wall[:, :])
            plw_t = const_pool.tile([P, NT], f32)
            nc.scalar.activation(out=plw_t[:], in_=w_t[:], func=ACTF.Ln)
            nc.vector.tensor_scalar_mul(plw_t[:], plw_t[:], p_t[:, 0:1])

            gw3 = gw_t[:].rearrange("p (t d) -> p t d", d=D)

            for w in range(NWIN_CORE):
                psA = psA_pool.tile([P, D + 1], f32, tag="psA")
                for gi in range(GPW):
                    g = w * GPW + gi
                    xs = xs_pool.tile([P, GROUP * (D + 1)], f32, tag="xs")
                    nc.sync.dma_start(xs[:], xp[g * P : (g + 1) * P, :])
                    xs3 = xs[:].rearrange("p (t d) -> p t d", d=D + 1)

                    xw = grp_pool.tile([P, GROUP * D], f32, tag="xw")
                    xw3 = xw[:].rearrange("p (t d) -> p t d", d=D)
                    if MULT_ON_GPSIMD:
                        nc.gpsimd.tensor_tensor(
                            out=xw3, in0=xs3[:, :, 0:D], in1=gw3, op=ALU.mult
                        )
                    else:
                        nc.vector.tensor_tensor(
                            out=xw3, in0=xs3[:, :, 0:D], in1=gw3, op=ALU.mult
                        )
                    logit = grp_pool.tile([P, GROUP], f32, tag="logit")
                    nc.vector.reduce_sum(out=logit[:], in_=xw3, axis=AX.X)
                    nc.vector.tensor_add(
                        logit[:], logit[:], plw_t[:, g * GROUP : (g + 1) * GROUP]
                    )
                    gex = grp_pool.tile([P, GROUP], f32, tag="gex")
                    nc.scalar.activation(
                        out=gex[:], in_=logit[:], func=ACTF.Exp, bias=gateb_t[:, 0:1]
                    )

                    for j in range(GROUP):
                        t_in_win = gi * GROUP + j
                        t_glob = g * GROUP + j
                        G = g_pool.tile([P, WIN], f32, tag="G")
                        nc.vector.tensor_scalar(
                            out=G[:],
                            in0=iota_t[:],
                            scalar1=idx_t[:, t_glob : t_glob + 1],
                            scalar2=gex[:, j : j + 1],
                            op0=ALU.is_equal,
                            op1=ALU.mult,
                        )
                        nc.tensor.matmul(
                            out=psA[:],
                            lhsT=G[:],
                            rhs=xs3[:, j, :],
                            start=(t_in_win == 0),
                            stop=(t_in_win == TPW - 1),
                        )

                # ---- phase 2: A @ msg_w / (denom+eps) + coef*msg_b ----
                sbA = ph2_pool.tile([P, D + 1], f32, tag="sbA")
                nc.vector.tensor_copy(sbA[:], psA[:])
                deno = ph2_pool.tile([P, 1], f32, tag="deno")
                nc.vector.tensor_scalar_add(deno[:], sbA[:, D : D + 1], EPS)
                rcp = ph2_pool.tile([P, 1], f32, tag="rcp")
                nc.vector.reciprocal(out=rcp[:], in_=deno[:])
                coef = ph2_pool.tile([P, 1], f32, tag="coef")
                nc.vector.tensor_tensor(
                    out=coef[:], in0=sbA[:, D : D + 1], in1=rcp[:], op=ALU.mult
                )

                psAT = ps2_pool.tile([P, D], f32, tag="AT")
                nc.tensor.transpose(out=psAT[:], in_=sbA[:, 0:D], identity=ident[:])
                sbAT = ph2_pool.tile([P, D], f32, tag="sbAT")
                nc.vector.tensor_copy(sbAT[:], psAT[:])
                ps2 = ps2_pool.tile([P, D], f32, tag="out2")
                nc.tensor.matmul(
                    out=ps2[:], lhsT=sbAT[:], rhs=msgw_t[:], start=True, stop=True
                )
                outsb = ph2_pool.tile([P, D], f32, tag="outsb")
                nc.vector.tensor_scalar_mul(outsb[:], ps2[:], rcp[:, 0:1])
                bterm = ph2_pool.tile([P, D], f32, tag="bterm")
                nc.vector.tensor_scalar_mul(bterm[:], msgb_t[:], coef[:, 0:1])
                nc.vector.tensor_add(outsb[:], outsb[:], bterm[:])
                nc.sync.dma_start(out[w * P : (w + 1) * P, :], outsb[:])

    nc.compile()
    _module_cache["nc"] = nc
    return nc


def _shard_inputs(x, idx, w):
    """Pad + reorder host arrays into the per-core device layouts."""
    n = idx.shape[0]
    bounds = np.searchsorted(idx, np.arange(0, S + 1, WIN)).astype(np.int64)
    counts = np.diff(bounds)
    if counts.max() > TPW * P:
        raise RuntimeError(f"window overflow: {counts.max()} > {TPW * P}")

    dest = np.arange(n, dtype=np.int64) + np.repeat(
        np.arange(NWIN, dtype=np.int64) * (TPW * P) - bounds[:-1], counts
    )

    xpad = np.zeros((NCORES * ROWS_CORE, D + 1), dtype=np.float32)
    xpad[:, D] = 1.0
    xpad[dest, 0:D] = x
    idxl = np.full(NCORES * ROWS_CORE, float(WIN + 7), dtype=np.float32)
    idxl[dest] = (idx - np.repeat(np.arange(NWIN, dtype=np.int64) * WIN, counts)).astype(
        np.float32
    )
    wpad = np.ones(NCORES * ROWS_CORE, dtype=np.float32)
    wpad[dest] = w

    # device layout: per core, per group: [128 partitions, GROUP tiles, ...]
    xdev = (
        xpad.reshape(NCORES, NG, GROUP, P, D + 1)
        .transpose(0, 1, 3, 2, 4)
        .reshape(NCORES, NG * P, GROUP * (D + 1))
    )
    # per-tile scalars as [P, NT] (tile-major along free dim)
    idxdev = np.ascontiguousarray(
        idxl.reshape(NCORES, NT, P).transpose(0, 2, 1)
    )
    wdev = np.ascontiguousarray(wpad.reshape(NCORES, NT, P).transpose(0, 2, 1))
    return xdev, idxdev, wdev


def _ensure_ntff_hook():
    """The image's antenv package lacks axon_hooks; shim it so trace=True
    can register the ctypes NTFF hook from trn_agent_boot."""
    try:
        from antenv.axon_hooks import get_axon_ntff_profile_hook  # noqa: F401

        return True
    except ImportError:
        pass
    try:
        import types

        import antenv
        from trn_agent_boot.trn_boot import _ntff_profile_via_ctypes

        mod = types.ModuleType("antenv.axon_hooks")
        _hook = [None]
        mod.set_axon_ntff_profile_hook = lambda h: _hook.__setitem__(0, h)
        mod.get_axon_ntff_profile_hook = lambda: _hook[0]
        sys.modules["antenv.axon_hooks"] = mod
        antenv.axon_hooks = mod
        mod.set_axon_ntff_profile_hook(
            _ntff_profile_via_ctypes("/opt/axon/libaxon_pjrt.so")
        )
        return True
    except Exception as e:  # degrade to untraced run
        print(f"ntff hook install failed: {type(e).__name__}: {e}")
        return False


def kernel(x, index, weights, gate_w, gate_b, msg_w, msg_b, pow_p):
    global LAST_EXEC_NS, LAST_RESULTS

    x = np.ascontiguousarray(np.asarray(x, dtype=np.float32))
    idx = np.asarray(index).astype(np.int64).ravel()
    w = np.asarray(weights, dtype=np.float32).ravel()
    gate_w = np.asarray(gate_w, dtype=np.float32).reshape(D)
    gate_b = np.asarray(gate_b, dtype=np.float32).reshape(1)
    msg_w = np.ascontiguousarray(np.asarray(msg_w, dtype=np.float32))
    msg_b = np.asarray(msg_b, dtype=np.float32).reshape(D)
    pow_p = np.asarray(pow_p, dtype=np.float32).reshape(1)

    if not np.all(idx[1:] >= idx[:-1]):
        perm = np.argsort(idx, kind="stable")
        idx = idx[perm]
        x = x[perm]
        w = w[perm]

    xdev, idxdev, wdev = _shard_inputs(x, idx, w)

    gwrep = np.tile(gate_w[None, :], (P, GROUP)).astype(np.float32)
    msgbrep = np.tile(msg_b[None, :], (P, 1)).astype(np.float32)
    gatebrep = np.full((P, 1), gate_b[0], dtype=np.float32)
    prep = np.full((P, 1), pow_p[0], dtype=np.float32)
    iota = np.tile(np.arange(WIN)[None, :], (P, 1))
    if IOTA_BF16:
        import ml_dtypes

        iota = iota.astype(ml_dtypes.bfloat16)
    else:
        iota = iota.astype(np.float32)

    nc = _build_module()
    from concourse.bass_utils import run_bass_kernel_spmd

    in_maps = []
    for c in range(NCORES):
        in_maps.append(
            {
                "xp": np.ascontiguousarray(xdev[c]),
                "idxall": idxdev[c],
                "wall": wdev[c],
                "gwrep": gwrep,
                "msgw": msg_w,
                "msgbrep": msgbrep,
                "gatebrep": gatebrep,
                "prep": prep,
                "iota": iota,
            }
        )

    trace = bool(os.environ.get("KERNEL_TRACE"))
    if trace:
        trace = _ensure_ntff_hook()
    res = run_bass_kernel_spmd(
        nc, in_maps, core_ids=list(range(NCORES)), trace=trace
    )
    LAST_RESULTS = res
    LAST_EXEC_NS = res.exec_time_ns

    out = np.concatenate([res.results[c]["out"] for c in range(NCORES)], axis=0)
    return out.astype(np.float32)


def kernel_numpy(x, index, weights, gate_w, gate_b, msg_w, msg_b, pow_p):
    """Host-side mirror of the device algorithm (debug only)."""
    x = np.asarray(x, dtype=np.float32)
    idx = np.asarray(index).astype(np.int64).ravel()
    w = np.asarray(weights, dtype=np.float32).ravel()
    gate = x @ np.asarray(gate_w, dtype=np.float32).reshape(D, 1)
    gate = gate[:, 0] + np.asarray(gate_b).reshape(1)[0]
    g = np.exp(gate + np.asarray(pow_p).reshape(1)[0] * np.log(w))
    A = np.zeros((S, D), dtype=np.float64)
    den = np.zeros(S, dtype=np.float64)
    np.add.at(A, idx, g[:, None] * x)
    np.add.at(den, idx, g)
    out = (A @ np.asarray(msg_w, dtype=np.float64)) / (den[:, None] + EPS)
    out = out + (den / (den + EPS))[:, None] * np.asarray(msg_b).reshape(1, D)
    return out.astype(np.float32)
